# revision 44
# baseline (speedup 1.0000x reference)
"""Two-layer GAT (PyG GATConv semantics) on 8 Trainium2 NeuronCores.

v1 strategy (node/graph parallel, descriptor-count-optimized):
- Nodes degree-sorted, dealt round-robin to 8 cores (dst sharding); per-core
  per-dst-group padded slot lists as in v0 (1% padding).
- LAYER 1 has NO indirect gathers: the host pre-gathers x into per-edge-slot
  order (x_edgeT, bf16), and the TensorEngine projects PER EDGE
  (x_e @ [W1|W1@Asrc]) straight into the slot-aligned SBUF tile. A second
  K=1 matmul adds -1000 to the a_src column of padded slots (mask row).
- a_dst terms for both layers are computed per OWN node into persistent SBUF
  (no DRAM round-trip): layer 1 from x_ownT @ (W1@Adst); layer 2 from the
  already-transposed layer-1 output (ot @ (W2@Adst2)) inside finish1.
- LAYER 2 aggregates alpha-weighted h1relu (64 dims, shared across heads)
  instead of the 256-dim h2: table2 rows are [h1relu | a_src2] (68 cols);
  W2 is applied AFTER aggregation via two accumulating PE matmuls against a
  head-rearranged W2 (out = mean_h (sum_e alpha_h h1relu_e) @ W2_h + b2).
  This cuts layer-2 gather traffic 3.25x and the table write 3.7x.
- Single AllGather exchanges transposed layer-1 outputs (as v0).
"""
import sys

sys.path.insert(0, "/opt/trn_rl_repo")

from contextlib import ExitStack

import numpy as np
import ml_dtypes

import concourse.bass as bass
import concourse.tile as tile
from concourse import mybir
import bass_rust as _bass_rust
from concourse.bass_utils import run_bass_kernel_spmd
from concourse.masks import make_identity

NC = 8
P = 128
HEADS = 4
NEG_SLOPE = 0.2
EPS = 1e-16
SENT_ASRC = -1000.0

F32 = mybir.dt.float32
BF16 = mybir.dt.bfloat16
I32 = mybir.dt.int32
BF_NP = ml_dtypes.bfloat16


def _host_prep(x, edge_index):
    n, in_ch = x.shape
    src = np.concatenate([np.asarray(edge_index[0]), np.arange(n, dtype=np.int64)])
    dst = np.concatenate([np.asarray(edge_index[1]), np.arange(n, dtype=np.int64)])
    deg = np.bincount(dst, minlength=n)

    order = np.argsort(-deg, kind="stable")  # order[rank] = node
    rank = np.empty(n, dtype=np.int64)
    rank[order] = np.arange(n)

    nloc = ((n + NC - 1) // NC + P - 1) // P * P  # local slots per core
    ntab = NC * nloc
    ng = nloc // P

    # rank r -> core r%NC, slot r//NC, table row t
    t_of = (rank % NC) * nloc + rank // NC  # per node

    td = t_of[dst]
    ts = t_of[src].astype(np.int32)
    c_e = td // nloc
    loc = td % nloc
    g_e = loc // P
    lane_e = loc % P

    key = (c_e * ng + g_e) * P + lane_e
    cnt = np.bincount(key, minlength=NC * ng * P).reshape(NC, ng, P)
    s_g = np.maximum(cnt.max(axis=(0, 2)), 1)  # padded slots per group
    s0 = np.zeros(ng, dtype=np.int64)
    s0[1:] = np.cumsum(s_g)[:-1]
    st = int(s_g.sum())

    sidx = np.argsort(key, kind="stable")
    ks = key[sidx]
    starts = np.searchsorted(ks, np.arange(NC * ng * P))
    slot = np.arange(len(ks)) - starts[ks]

    idx_arr = np.full((NC, P, st), ntab, dtype=np.int32)  # sentinel row id
    col = s0[g_e[sidx]] + slot
    idx_arr[c_e[sidx], lane_e[sidx], col] = ts[sidx]

    # per-core edge-expanded x (transposed, bf16) in slot-major order:
    # column j*128+lane holds x[src] of (lane, slot j); padded slots get 0
    # and maskP = -1000 (added onto the a_src columns after projection).
    xf = np.asarray(x, dtype=np.float32)
    x_edgeT = np.empty((NC, in_ch, st * P), dtype=BF_NP)
    maskP = np.zeros((NC, P, st), dtype=np.float32)
    for c in range(NC):
        arrF = idx_arr[c].T.reshape(-1)  # [st*P], slot-major, lane fastest
        pad = arrF == ntab
        t_real = np.where(pad, 0, arrF).astype(np.int64)
        ranks = (t_real % nloc) * NC + t_real // nloc
        nodes = order[np.minimum(ranks, n - 1)]
        nodes[pad] = 0
        xe = xf[nodes]
        xe[pad] = 0.0
        x_edgeT[c] = np.ascontiguousarray(xe.T).astype(BF_NP)
        maskP[c] = np.where(idx_arr[c] == ntab, SENT_ASRC, 0.0)

    # own-node x (transposed, bf16): column loc holds x of own node at loc
    x_ownT = np.zeros((NC, in_ch, nloc), dtype=BF_NP)
    for c in range(NC):
        locs = np.arange(nloc)
        rk = locs * NC + c
        valid = rk < n
        xo = np.zeros((nloc, in_ch), dtype=np.float32)
        xo[valid] = xf[order[rk[valid]]]
        x_ownT[c] = np.ascontiguousarray(xo.T).astype(BF_NP)

    meta = {
        "n": n,
        "nloc": nloc,
        "ntab": ntab,
        "ng": ng,
        "st": st,
        "s_g": s_g.astype(np.int64),
        "s0": s0,
        "order": order,
    }
    return x_edgeT, maskP, x_ownT, idx_arr, meta


def _build_program(meta, in_ch, hid, out_ch):
    """One SPMD program for all 8 cores."""
    ntab, ng, st, nloc = meta["ntab"], meta["ng"], meta["st"], meta["nloc"]
    s_g, s0 = meta["s_g"], meta["s0"]
    f1 = HEADS * hid       # 64
    f2 = HEADS * out_ch    # 256
    smax = int(s_g.max())

    nc = bass.Bass(num_devices=NC)

    x_edgeT_d = nc.declare_dram_parameter("x_edgeT", [in_ch, st * P], BF16, isOutput=False)
    maskp_d = nc.declare_dram_parameter("maskp", [P, st], F32, isOutput=False)
    x_ownT_d = nc.declare_dram_parameter("x_ownT", [in_ch, nloc], BF16, isOutput=False)
    idx_d = nc.declare_dram_parameter("idx", [P, st], I32, isOutput=False)
    w1_d = nc.declare_dram_parameter("w1", [in_ch, f1], F32, isOutput=False)
    asrc1_d = nc.declare_dram_parameter("asrc1", [1, f1], F32, isOutput=False)
    adst1_d = nc.declare_dram_parameter("adst1", [1, f1], F32, isOutput=False)
    b1_d = nc.declare_dram_parameter("b1", [1, f1], F32, isOutput=False)
    w2_d = nc.declare_dram_parameter("w2", [f1, f2], F32, isOutput=False)
    asrc2_d = nc.declare_dram_parameter("asrc2", [1, f2], F32, isOutput=False)
    adst2_d = nc.declare_dram_parameter("adst2", [1, f2], F32, isOutput=False)
    b2_d = nc.declare_dram_parameter("b2", [1, out_ch], F32, isOutput=False)
    out2_d = nc.declare_dram_parameter("out2", [nloc, out_ch], F32, isOutput=True)

    h2cat = nc.dram_tensor("h2cat", [ntab + 1, f1 + 4], F32)
    out1t = nc.dram_tensor("out1t", [f1, nloc], F32)
    ag_out = nc.dram_tensor("ag_out", [NC * f1, nloc], F32, addr_space="Shared")

    def bcast_row(dram_t, width):
        return bass.AP(
            tensor=dram_t[:].tensor,
            offset=dram_t[:].offset,
            ap=[[0, P], [1, width]],
        )

    with tile.TileContext(nc) as tc, ExitStack() as ctx:
        const = ctx.enter_context(tc.tile_pool(name="const", bufs=1))
        pool = ctx.enter_context(tc.tile_pool(name="work", bufs=3))
        psum1_cm = tc.tile_pool(name="psum1", bufs=1, space="PSUM")
        psum1 = psum1_cm.__enter__()

        # ---- constants ----
        w1_sb = const.tile([in_ch, f1], F32)
        nc.sync.dma_start(out=w1_sb[:], in_=w1_d[:, :])
        w2_sb = const.tile([f1, f2], F32)
        nc.sync.dma_start(out=w2_sb[:], in_=w2_d[:, :])
        b1_b = const.tile([P, f1], F32)
        nc.sync.dma_start(out=b1_b[:], in_=bcast_row(b1_d, f1))
        b2_b = const.tile([P, out_ch], F32)
        nc.sync.dma_start(out=b2_b[:], in_=bcast_row(b2_d, out_ch))
        ident = const.tile([P, P], F32)
        make_identity(nc, ident[:])
        idx_sb = const.tile([P, st], I32)
        nc.sync.dma_start(out=idx_sb[:], in_=idx_d[:, :])

        # W2 rearranged for the post-aggregation matmul:
        # W2r[h*64+d, j] = W2[d, h*64+j]; stored as two 128-partition halves.
        w2r_lo = const.tile([2 * f1, out_ch], F32)
        nc.sync.dma_start(out=w2r_lo[0:f1, :], in_=w2_d[:, 0:out_ch])
        nc.sync.dma_start(out=w2r_lo[f1 : 2 * f1, :], in_=w2_d[:, out_ch : 2 * out_ch])
        w2r_hi = const.tile([2 * f1, out_ch], F32)
        nc.sync.dma_start(out=w2r_hi[0:f1, :], in_=w2_d[:, 2 * out_ch : 3 * out_ch])
        nc.sync.dma_start(out=w2r_hi[f1 : 2 * f1, :], in_=w2_d[:, 3 * out_ch : 4 * out_ch])

        # layer-2 sentinel row: h=0, a_src2=-1000
        sent2 = const.tile([1, f1 + 4], F32)
        nc.vector.memset(sent2[:], 0.0)
        nc.vector.memset(sent2[:, f1 : f1 + 4], SENT_ASRC)
        nc.sync.dma_start(out=h2cat[ntab : ntab + 1, :], in_=sent2[:])

        # layer-1 pad mask: -1000 per padded (lane, slot), added onto a_src
        maskp_sb = const.tile([P, st], F32)
        nc.sync.dma_start(out=maskp_sb[:], in_=maskp_d[:, :])

        # ---- attention matrices: block-diag A[h*ch+c, h] = att[h, c] ----
        def build_attmat(att_d, fdim, tag):
            ch = fdim // HEADS
            chunks = []
            for k0 in range(0, fdim, P):
                rows = min(P, fdim - k0)
                a_sb = const.tile([rows, 4], F32, tag=f"{tag}_{k0}")
                nc.vector.memset(a_sb[:], 0.0)
                for h in range(HEADS):
                    lo, hi = h * ch, (h + 1) * ch
                    lo2, hi2 = max(lo, k0), min(hi, k0 + rows)
                    if lo2 < hi2:
                        nc.sync.dma_start(
                            out=a_sb[lo2 - k0 : hi2 - k0, h : h + 1],
                            in_=att_d[0:1, lo2:hi2],
                        )
                chunks.append(a_sb)
            return chunks

        as1_m = build_attmat(asrc1_d, f1, "as1m")
        ad1_m = build_attmat(adst1_d, f1, "ad1m")
        as2_m = build_attmat(asrc2_d, f2, "as2m")
        ad2_m = build_attmat(adst2_d, f2, "ad2m")

        # W1 @ A (contraction over f1=64): lhsT = W1^T via PE transpose.
        w1t_ps = psum1.tile([f1, in_ch], F32, tag="prep_t")
        nc.tensor.transpose(out=w1t_ps[:], in_=w1_sb[:], identity=ident[:])
        w1t = const.tile([f1, in_ch], F32)
        nc.vector.tensor_copy(out=w1t[:], in_=w1t_ps[:])
        w1as_ps = psum1.tile([in_ch, 4], F32, tag="prep_a")
        w1ad_ps = psum1.tile([in_ch, 4], F32, tag="prep_b")
        nc.tensor.matmul(out=w1as_ps[:], lhsT=w1t[:], rhs=as1_m[0][:], start=True, stop=True)
        nc.tensor.matmul(out=w1ad_ps[:], lhsT=w1t[:], rhs=ad1_m[0][:], start=True, stop=True)

        # w1aug_e (bf16): [W1 | W1@Asrc1] for the per-edge projection
        w1aug_f = const.tile([in_ch, f1 + 4], F32)
        nc.vector.tensor_copy(out=w1aug_f[:, 0:f1], in_=w1_sb[:])
        nc.vector.tensor_copy(out=w1aug_f[:, f1 : f1 + 4], in_=w1as_ps[:])
        w1aug_e = const.tile([in_ch, f1 + 4], BF16)
        nc.vector.tensor_copy(out=w1aug_e[:], in_=w1aug_f[:])
        wad1_bf = const.tile([in_ch, 4], BF16)
        nc.vector.tensor_copy(out=wad1_bf[:], in_=w1ad_ps[:])

        # W2 @ A2 (contraction over f2=256, split into K=128 halves).
        w2as_ps = psum1.tile([f1, 4], F32, tag="prep_a")
        w2ad_ps = psum1.tile([f1, 4], F32, tag="prep_b")
        nkh = f2 // P
        for kh in range(nkh):
            w2t_ps = psum1.tile([P, f1], F32, tag="prep_t")
            nc.tensor.transpose(
                out=w2t_ps[:], in_=w2_sb[:, kh * P : (kh + 1) * P],
                identity=ident[0:f1, 0:f1],
            )
            w2t = pool.tile([P, f1], F32, tag="w2t_sb")
            nc.vector.tensor_copy(out=w2t[:], in_=w2t_ps[:])
            nc.tensor.matmul(
                out=w2as_ps[:], lhsT=w2t[:], rhs=as2_m[kh][:],
                start=(kh == 0), stop=(kh == nkh - 1),
            )
            nc.tensor.matmul(
                out=w2ad_ps[:], lhsT=w2t[:], rhs=ad2_m[kh][:],
                start=(kh == 0), stop=(kh == nkh - 1),
            )
        # waug2p: [I_64 | W2@Asrc2] (rhs for table-2 row building)
        waug2p = const.tile([f1, f1 + 4], F32)
        nc.vector.tensor_copy(out=waug2p[:, 0:f1], in_=ident[0:f1, 0:f1])
        nc.vector.tensor_copy(out=waug2p[:, f1 : f1 + 4], in_=w2as_ps[:])
        wad2_sb = const.tile([f1, 4], F32)
        nc.vector.tensor_copy(out=wad2_sb[:], in_=w2ad_ps[:])
        psum1_cm.__exit__(None, None, None)

        psum = ctx.enter_context(tc.tile_pool(name="psum", bufs=2, space="PSUM"))
        psumo = ctx.enter_context(tc.tile_pool(name="psumo", bufs=1, space="PSUM"))
        psumt = ctx.enter_context(tc.tile_pool(name="psumt", bufs=1, space="PSUM"))

        # persistent per-own-node a_dst values (both layers)
        adst1_own = const.tile([P, ng, 4], F32)
        adst2_own = const.tile([P, ng, 4], F32)

        # ---- P1own: layer-1 a_dst for own nodes ----
        QO = 4
        for g0 in range(0, ng, QO):
            q = min(QO, ng - g0)
            xo = pool.tile([in_ch, QO * P], BF16, tag="p1o_x")
            nc.sync.dma_start(out=xo[:, : q * P], in_=x_ownT_d[:, g0 * P : (g0 + q) * P])
            for k in range(q):
                ps = psum.tile([P, 4], F32, tag="s4")
                nc.tensor.matmul(
                    out=ps[:], lhsT=xo[:, k * P : (k + 1) * P], rhs=wad1_bf[:],
                    start=True, stop=True,
                )
                nc.vector.tensor_copy(out=adst1_own[:, g0 + k, :], in_=ps[:])

        def edge_weights(t_sb, adst_own, g, sg, tag):
            """w = exp(leaky_relu(a_src + a_dst)) per (lane, slot, head);
            den[lane, head] accumulates the softmax denominators."""
            w_sb = pool.tile([P, smax, 4], F32, tag=f"w{tag}")
            den = pool.tile([P, 4], F32, tag=f"den{tag}")
            adst_g = adst_own[:, g, :]
            adst_bc = bass.AP(
                tensor=adst_g.tensor,
                offset=adst_g.offset,
                ap=[adst_g.ap[0], [0, sg], adst_g.ap[1]],
            )
            nc.vector.tensor_tensor(
                out=w_sb[:, :sg, :],
                in0=t_sb[:, :sg, f1 : f1 + 4],
                in1=adst_bc,
                op=mybir.AluOpType.add,
            )
            nc.vector.scalar_tensor_tensor(
                out=w_sb[:, :sg, :],
                in0=w_sb[:, :sg, :],
                scalar=NEG_SLOPE,
                in1=w_sb[:, :sg, :],
                op0=mybir.AluOpType.mult,
                op1=mybir.AluOpType.max,
            )
            for h in range(HEADS):
                nc.scalar.activation(
                    out=w_sb[:, :sg, h],
                    in_=w_sb[:, :sg, h],
                    func=mybir.ActivationFunctionType.Exp,
                    accum_out=den[:, h : h + 1],
                )
            return w_sb, den

        # ---- A1: per-edge projection + layer-1 aggregation -> out1t ----
        with tc.tile_pool(name="gather1", bufs=2) as gpool1:
            for g in range(ng):
                sg = int(s_g[g])
                base = int(s0[g])
                xe = gpool1.tile([in_ch, smax * P], BF16, tag="xe")
                nc.sync.dma_start(
                    out=xe[:, : sg * P],
                    in_=x_edgeT_d[:, base * P : (base + sg) * P],
                )
                t_sb = gpool1.tile([P, smax, f1 + 4], F32, tag="t1")
                for j in range(sg):
                    ps = psum.tile([P, f1 + 4], F32, tag="e1")
                    nc.tensor.matmul(
                        out=ps[:], lhsT=xe[:, j * P : (j + 1) * P], rhs=w1aug_e[:],
                        start=True, stop=True,
                    )
                    nc.vector.tensor_copy(out=t_sb[:, j, :], in_=ps[:])
                msl = maskp_sb[:, base : base + sg]
                mbc = bass.AP(
                    tensor=msl.tensor,
                    offset=msl.offset,
                    ap=[msl.ap[0], msl.ap[1], [0, 4]],
                )
                nc.vector.tensor_tensor(
                    out=t_sb[:, :sg, f1 : f1 + 4],
                    in0=t_sb[:, :sg, f1 : f1 + 4],
                    in1=mbc,
                    op=mybir.AluOpType.add,
                )

                w_sb, den = edge_weights(t_sb, adst1_own, g, sg, "1")
                hv = t_sb[:, :sg, 0:f1].rearrange("p g (h c) -> p g h c", h=HEADS)
                nc.vector.tensor_tensor(
                    out=hv,
                    in0=hv,
                    in1=w_sb[:, :sg, :].to_broadcast([P, sg, HEADS, f1 // HEADS]),
                    op=mybir.AluOpType.mult,
                )
                u = pool.tile([P, f1], F32, tag="u1")
                nc.vector.tensor_reduce(
                    out=u[:],
                    in_=t_sb[:, :sg, 0:f1].rearrange("p g f -> p f g"),
                    axis=mybir.AxisListType.X,
                    op=mybir.AluOpType.add,
                )
                r = pool.tile([P, 4], F32, tag="r1")
                nc.vector.tensor_scalar_add(out=r[:], in0=den[:], scalar1=EPS)
                nc.vector.reciprocal(out=r[:], in_=r[:])

                # finish1: normalize + bias + relu -> out1t; also a_dst2
                o = pool.tile([P, f1], F32, tag="o1")
                nc.vector.tensor_tensor(
                    out=o[:].rearrange("p (h c) -> p h c", h=HEADS),
                    in0=u[:].rearrange("p (h c) -> p h c", h=HEADS),
                    in1=r[:].to_broadcast([P, HEADS, f1 // HEADS]),
                    op=mybir.AluOpType.mult,
                )
                nc.vector.tensor_add(out=o[:], in0=o[:], in1=b1_b[:])
                nc.vector.tensor_scalar_max(out=o[:], in0=o[:], scalar1=0.0)
                tps = psumt.tile([f1, P], F32, tag="tr_ps")
                nc.tensor.transpose(out=tps[:], in_=o[:], identity=ident[:])
                ot = pool.tile([f1, P], F32, tag="o1t")
                nc.vector.tensor_copy(out=ot[:], in_=tps[:])
                nc.sync.dma_start(out=out1t[:, g * P : (g + 1) * P], in_=ot[:])
                psa = psum.tile([P, 4], F32, tag="s4")
                nc.tensor.matmul(
                    out=psa[:], lhsT=ot[:], rhs=wad2_sb[:], start=True, stop=True
                )
                nc.vector.tensor_copy(out=adst2_own[:, g, :], in_=psa[:])

        # ---- AG: exchange layer-1 outputs ----
        nc.gpsimd.collective_compute(
            "AllGather",
            mybir.AluOpType.bypass,
            replica_groups=[list(range(NC))],
            ins=[out1t[:]],
            outs=[ag_out[:]],
        )

        # ---- P2: table-2 rows [h1relu | a_src2] for ALL nodes ----
        QMAX = 4
        for c_i in range(NC):
            for g0 in range(0, ng, QMAX):
                q = min(QMAX, ng - g0)
                lt = pool.tile([f1, QMAX * P], F32, tag="p2_l")
                nc.sync.dma_start(
                    out=lt[:, : q * P],
                    in_=ag_out[c_i * f1 : (c_i + 1) * f1, g0 * P : (g0 + q) * P],
                )
                nc.vector.tensor_scalar_max(
                    out=lt[:, : q * P], in0=lt[:, : q * P], scalar1=0.0
                )
                hc = pool.tile([P, QMAX, f1 + 4], F32, tag="p2_hc")
                for k in range(q):
                    ps = psum.tile([P, f1 + 4], F32, tag="e1")
                    nc.tensor.matmul(
                        out=ps[:], lhsT=lt[:, k * P : (k + 1) * P], rhs=waug2p[:],
                        start=True, stop=True,
                    )
                    nc.vector.tensor_copy(out=hc[:, k, :], in_=ps[:])
                r0 = (c_i * ng + g0) * P
                nc.sync.dma_start(
                    out=h2cat[r0 : r0 + q * P, :].rearrange("(k p) f -> p k f", k=q),
                    in_=hc[:, :q, :],
                )

        # ---- A2: layer-2 aggregation (gather 68-col rows, 4-head reduce) ----
        with tc.tile_pool(name="gather2", bufs=2) as gpool2:
            for g in range(ng):
                sg = int(s_g[g])
                base = int(s0[g])
                t2 = gpool2.tile([P, smax, f1 + 4], F32, tag="t2")
                for j in range(sg):
                    nc.gpsimd.indirect_dma_start(
                        out=t2[:, j, :],
                        out_offset=None,
                        in_=h2cat[:],
                        in_offset=bass.IndirectOffsetOnAxis(
                            ap=idx_sb[:, base + j : base + j + 1], axis=0
                        ),
                    )
                w_sb, den = edge_weights(t2, adst2_own, g, sg, "2")
                u = pool.tile([P, HEADS, f1], F32, tag="u2")
                for h in range(HEADS):
                    pr = pool.tile([P, smax, f1], F32, tag="pr2")
                    wsl = w_sb[:, :sg, h : h + 1]
                    whb = bass.AP(
                        tensor=wsl.tensor,
                        offset=wsl.offset,
                        ap=[wsl.ap[0], wsl.ap[1], [0, f1]],
                    )
                    nc.vector.tensor_tensor(
                        out=pr[:, :sg, :],
                        in0=t2[:, :sg, 0:f1],
                        in1=whb,
                        op=mybir.AluOpType.mult,
                    )
                    nc.vector.tensor_reduce(
                        out=u[:, h, :],
                        in_=pr[:, :sg, :].rearrange("p g f -> p f g"),
                        axis=mybir.AxisListType.X,
                        op=mybir.AluOpType.add,
                    )
                # finish2: scale by recip(den)/HEADS, apply W2, +b2
                r = pool.tile([P, 4], F32, tag="r2")
                nc.vector.tensor_scalar_add(out=r[:], in0=den[:], scalar1=EPS)
                nc.vector.reciprocal(out=r[:], in_=r[:])
                nc.vector.tensor_scalar_mul(out=r[:], in0=r[:], scalar1=1.0 / HEADS)
                agg = pool.tile([P, HEADS, f1], F32, tag="agg2")
                rb = bass.AP(
                    tensor=r[:].tensor,
                    offset=r[:].offset,
                    ap=[r[:].ap[0], r[:].ap[1], [0, f1]],
                )
                nc.vector.tensor_tensor(
                    out=agg[:], in0=u[:], in1=rb, op=mybir.AluOpType.mult
                )
                aggf = agg[:].rearrange("p h f -> p (h f)")
                ops = psumo.tile([P, out_ch], F32, tag="o2ps")
                for half in range(2):
                    tph = psumt.tile([P, P], F32, tag="tr2")
                    nc.tensor.transpose(
                        out=tph[:],
                        in_=aggf[:, half * P : (half + 1) * P],
                        identity=ident[:],
                    )
                    aggT = pool.tile([P, P], F32, tag=f"aggT{half}")
                    nc.vector.tensor_copy(out=aggT[:], in_=tph[:])
                    nc.tensor.matmul(
                        out=ops[:],
                        lhsT=aggT[:],
                        rhs=(w2r_lo if half == 0 else w2r_hi)[:],
                        start=(half == 0),
                        stop=(half == 1),
                    )
                o2 = pool.tile([P, out_ch], F32, tag="o2")
                nc.vector.tensor_tensor(
                    out=o2[:], in0=ops[:], in1=b2_b[:], op=mybir.AluOpType.add
                )
                nc.sync.dma_start(out=out2_d[g * P : (g + 1) * P, :], in_=o2[:])

    _bass_rust.generate_event_semaphores(nc)
    return nc


def kernel(x, edge_index, W1, att_src1, att_dst1, b1, W2, att_src2, att_dst2, b2, trace=False):
    x = np.asarray(x, dtype=np.float32)
    edge_index = np.asarray(edge_index)
    in_ch = x.shape[1]
    hid = np.asarray(att_src1).shape[1]
    out_ch = np.asarray(att_src2).shape[1]
    f1, f2 = HEADS * hid, HEADS * out_ch

    x_edgeT, maskP, x_ownT, idx_arr, meta = _host_prep(x, edge_index)
    nc = _build_program(meta, in_ch, hid, out_ch)

    common = {
        "w1": np.asarray(W1, dtype=np.float32),
        "asrc1": np.asarray(att_src1, dtype=np.float32).reshape(1, f1),
        "adst1": np.asarray(att_dst1, dtype=np.float32).reshape(1, f1),
        "b1": np.asarray(b1, dtype=np.float32).reshape(1, f1),
        "w2": np.asarray(W2, dtype=np.float32),
        "asrc2": np.asarray(att_src2, dtype=np.float32).reshape(1, f2),
        "adst2": np.asarray(att_dst2, dtype=np.float32).reshape(1, f2),
        "b2": np.asarray(b2, dtype=np.float32).reshape(1, out_ch),
    }
    in_maps = [
        {
            **common,
            "x_edgeT": x_edgeT[c],
            "maskp": maskP[c],
            "x_ownT": x_ownT[c],
            "idx": np.ascontiguousarray(idx_arr[c]),
        }
        for c in range(NC)
    ]
    if trace:
        import axon_prof

        axon_prof.install()
    r = run_bass_kernel_spmd(nc, in_maps, list(range(NC)), trace=trace)

    n, nloc, order = meta["n"], meta["nloc"], meta["order"]
    out = np.zeros((n, out_ch), dtype=np.float32)
    for c in range(NC):
        j = np.arange(nloc)
        rk = j * NC + c
        valid = rk < n
        out[order[rk[valid]]] = r.results[c]["out2"][valid]
    if trace:
        return out, r
    return out


# revision 45
# speedup vs baseline: 1.1982x; 1.1982x over previous
"""Two-layer GAT (PyG GATConv semantics) on 8 Trainium2 NeuronCores.

v1 strategy (node/graph parallel, descriptor-count-optimized):
- Nodes degree-sorted, dealt round-robin to 8 cores (dst sharding); per-core
  per-dst-group padded slot lists as in v0 (1% padding).
- LAYER 1 has NO indirect gathers: the host pre-gathers x into per-edge-slot
  order (x_edgeT, bf16), and the TensorEngine projects PER EDGE
  (x_e @ [W1|W1@Asrc]) straight into the slot-aligned SBUF tile. A second
  K=1 matmul adds -1000 to the a_src column of padded slots (mask row).
- a_dst terms for both layers are computed per OWN node into persistent SBUF
  (no DRAM round-trip): layer 1 from x_ownT @ (W1@Adst); layer 2 from the
  already-transposed layer-1 output (ot @ (W2@Adst2)) inside finish1.
- LAYER 2 aggregates alpha-weighted h1relu (64 dims, shared across heads)
  instead of the 256-dim h2: table2 rows are [h1relu | a_src2] (68 cols);
  W2 is applied AFTER aggregation via two accumulating PE matmuls against a
  head-rearranged W2 (out = mean_h (sum_e alpha_h h1relu_e) @ W2_h + b2).
  This cuts layer-2 gather traffic 3.25x and the table write 3.7x.
- Single AllGather exchanges transposed layer-1 outputs (as v0).
"""
import sys

sys.path.insert(0, "/opt/trn_rl_repo")

from contextlib import ExitStack

import numpy as np
import ml_dtypes

import concourse.bass as bass
import concourse.tile as tile
from concourse import mybir
import bass_rust as _bass_rust
from concourse.bass_utils import run_bass_kernel_spmd
from concourse.masks import make_identity

NC = 8
P = 128
HEADS = 4
NEG_SLOPE = 0.2
EPS = 1e-16
SENT_ASRC = -1000.0

F32 = mybir.dt.float32
BF16 = mybir.dt.bfloat16
I32 = mybir.dt.int32
BF_NP = ml_dtypes.bfloat16


def _host_prep(x, edge_index):
    n, in_ch = x.shape
    src = np.concatenate([np.asarray(edge_index[0]), np.arange(n, dtype=np.int64)])
    dst = np.concatenate([np.asarray(edge_index[1]), np.arange(n, dtype=np.int64)])
    deg = np.bincount(dst, minlength=n)

    order = np.argsort(-deg, kind="stable")  # order[rank] = node
    rank = np.empty(n, dtype=np.int64)
    rank[order] = np.arange(n)

    nloc = ((n + NC - 1) // NC + P - 1) // P * P  # local slots per core
    ntab = NC * nloc
    ng = nloc // P

    # rank r -> core r%NC, slot r//NC, table row t
    t_of = (rank % NC) * nloc + rank // NC  # per node

    td = t_of[dst]
    ts = t_of[src].astype(np.int32)
    c_e = td // nloc
    loc = td % nloc
    g_e = loc // P
    lane_e = loc % P

    key = (c_e * ng + g_e) * P + lane_e
    cnt = np.bincount(key, minlength=NC * ng * P).reshape(NC, ng, P)
    s_g = np.maximum(cnt.max(axis=(0, 2)), 1)  # padded slots per group
    s0 = np.zeros(ng, dtype=np.int64)
    s0[1:] = np.cumsum(s_g)[:-1]
    st = int(s_g.sum())

    sidx = np.argsort(key, kind="stable")
    ks = key[sidx]
    starts = np.searchsorted(ks, np.arange(NC * ng * P))
    slot = np.arange(len(ks)) - starts[ks]

    idx_arr = np.full((NC, P, st), ntab, dtype=np.int32)  # sentinel row id
    col = s0[g_e[sidx]] + slot
    idx_arr[c_e[sidx], lane_e[sidx], col] = ts[sidx]

    # per-core edge-expanded x (transposed, bf16) in slot-major order:
    # column j*128+lane holds x[src] of (lane, slot j); padded slots get 0
    # and maskP = -1000 (added onto the a_src columns after projection).
    xf = np.asarray(x, dtype=np.float32)
    x_edgeT = np.empty((NC, in_ch, st * P), dtype=BF_NP)
    maskP = np.zeros((NC, P, st), dtype=np.float32)
    for c in range(NC):
        arrF = idx_arr[c].T.reshape(-1)  # [st*P], slot-major, lane fastest
        pad = arrF == ntab
        t_real = np.where(pad, 0, arrF).astype(np.int64)
        ranks = (t_real % nloc) * NC + t_real // nloc
        nodes = order[np.minimum(ranks, n - 1)]
        nodes[pad] = 0
        xe = xf[nodes]
        xe[pad] = 0.0
        x_edgeT[c] = np.ascontiguousarray(xe.T).astype(BF_NP)
        maskP[c] = np.where(idx_arr[c] == ntab, SENT_ASRC, 0.0)

    # own-node x (transposed, bf16): column loc holds x of own node at loc
    x_ownT = np.zeros((NC, in_ch, nloc), dtype=BF_NP)
    for c in range(NC):
        locs = np.arange(nloc)
        rk = locs * NC + c
        valid = rk < n
        xo = np.zeros((nloc, in_ch), dtype=np.float32)
        xo[valid] = xf[order[rk[valid]]]
        x_ownT[c] = np.ascontiguousarray(xo.T).astype(BF_NP)

    meta = {
        "n": n,
        "nloc": nloc,
        "ntab": ntab,
        "ng": ng,
        "st": st,
        "s_g": s_g.astype(np.int64),
        "s0": s0,
        "order": order,
    }
    return x_edgeT, maskP, x_ownT, idx_arr, meta


def _build_program(meta, in_ch, hid, out_ch):
    """One SPMD program for all 8 cores."""
    ntab, ng, st, nloc = meta["ntab"], meta["ng"], meta["st"], meta["nloc"]
    s_g, s0 = meta["s_g"], meta["s0"]
    f1 = HEADS * hid       # 64
    f2 = HEADS * out_ch    # 256
    smax = int(s_g.max())

    nc = bass.Bass(num_devices=NC)

    x_edgeT_d = nc.declare_dram_parameter("x_edgeT", [in_ch, st * P], BF16, isOutput=False)
    maskp_d = nc.declare_dram_parameter("maskp", [P, st], F32, isOutput=False)
    x_ownT_d = nc.declare_dram_parameter("x_ownT", [in_ch, nloc], BF16, isOutput=False)
    idx_d = nc.declare_dram_parameter("idx", [P, st], I32, isOutput=False)
    w1_d = nc.declare_dram_parameter("w1", [in_ch, f1], F32, isOutput=False)
    asrc1_d = nc.declare_dram_parameter("asrc1", [1, f1], F32, isOutput=False)
    adst1_d = nc.declare_dram_parameter("adst1", [1, f1], F32, isOutput=False)
    b1_d = nc.declare_dram_parameter("b1", [1, f1], F32, isOutput=False)
    w2_d = nc.declare_dram_parameter("w2", [f1, f2], F32, isOutput=False)
    asrc2_d = nc.declare_dram_parameter("asrc2", [1, f2], F32, isOutput=False)
    adst2_d = nc.declare_dram_parameter("adst2", [1, f2], F32, isOutput=False)
    b2_d = nc.declare_dram_parameter("b2", [1, out_ch], F32, isOutput=False)
    out2_d = nc.declare_dram_parameter("out2", [nloc, out_ch], F32, isOutput=True)

    h2cat = nc.dram_tensor("h2cat", [ntab + 1, f1 + 4], F32)
    out1t = nc.dram_tensor("out1t", [f1, nloc], F32)
    ag_out = nc.dram_tensor("ag_out", [NC * f1, nloc], F32, addr_space="Shared")

    def bcast_row(dram_t, width):
        return bass.AP(
            tensor=dram_t[:].tensor,
            offset=dram_t[:].offset,
            ap=[[0, P], [1, width]],
        )

    with tile.TileContext(nc) as tc, ExitStack() as ctx:
        const = ctx.enter_context(tc.tile_pool(name="const", bufs=1))
        pool = ctx.enter_context(tc.tile_pool(name="work", bufs=3))
        psum1_cm = tc.tile_pool(name="psum1", bufs=1, space="PSUM")
        psum1 = psum1_cm.__enter__()

        # ---- constants ----
        w1_sb = const.tile([in_ch, f1], F32)
        nc.sync.dma_start(out=w1_sb[:], in_=w1_d[:, :])
        w2_sb = const.tile([f1, f2], F32)
        nc.sync.dma_start(out=w2_sb[:], in_=w2_d[:, :])
        b1_b = const.tile([P, f1], F32)
        nc.sync.dma_start(out=b1_b[:], in_=bcast_row(b1_d, f1))
        b2_b = const.tile([P, out_ch], F32)
        nc.sync.dma_start(out=b2_b[:], in_=bcast_row(b2_d, out_ch))
        ident = const.tile([P, P], F32)
        make_identity(nc, ident[:])
        idx_sb = const.tile([P, st], I32)
        nc.sync.dma_start(out=idx_sb[:], in_=idx_d[:, :])

        # W2 rearranged for the post-aggregation matmul:
        # W2r[h*64+d, j] = W2[d, h*64+j]; stored as two 128-partition halves.
        w2r_lo = const.tile([2 * f1, out_ch], F32)
        nc.sync.dma_start(out=w2r_lo[0:f1, :], in_=w2_d[:, 0:out_ch])
        nc.sync.dma_start(out=w2r_lo[f1 : 2 * f1, :], in_=w2_d[:, out_ch : 2 * out_ch])
        w2r_hi = const.tile([2 * f1, out_ch], F32)
        nc.sync.dma_start(out=w2r_hi[0:f1, :], in_=w2_d[:, 2 * out_ch : 3 * out_ch])
        nc.sync.dma_start(out=w2r_hi[f1 : 2 * f1, :], in_=w2_d[:, 3 * out_ch : 4 * out_ch])

        # layer-2 sentinel row: h=0, a_src2=-1000
        sent2 = const.tile([1, f1 + 4], F32)
        nc.vector.memset(sent2[:], 0.0)
        nc.vector.memset(sent2[:, f1 : f1 + 4], SENT_ASRC)
        nc.sync.dma_start(out=h2cat[ntab : ntab + 1, :], in_=sent2[:])

        # layer-1 pad mask: -1000 per padded (lane, slot), added onto a_src
        maskp_sb = const.tile([P, st], F32)
        nc.sync.dma_start(out=maskp_sb[:], in_=maskp_d[:, :])

        # ---- attention matrices: block-diag A[h*ch+c, h] = att[h, c] ----
        def build_attmat(att_d, fdim, tag):
            ch = fdim // HEADS
            chunks = []
            for k0 in range(0, fdim, P):
                rows = min(P, fdim - k0)
                a_sb = const.tile([rows, 4], F32, tag=f"{tag}_{k0}")
                nc.vector.memset(a_sb[:], 0.0)
                for h in range(HEADS):
                    lo, hi = h * ch, (h + 1) * ch
                    lo2, hi2 = max(lo, k0), min(hi, k0 + rows)
                    if lo2 < hi2:
                        nc.sync.dma_start(
                            out=a_sb[lo2 - k0 : hi2 - k0, h : h + 1],
                            in_=att_d[0:1, lo2:hi2],
                        )
                chunks.append(a_sb)
            return chunks

        as1_m = build_attmat(asrc1_d, f1, "as1m")
        ad1_m = build_attmat(adst1_d, f1, "ad1m")
        as2_m = build_attmat(asrc2_d, f2, "as2m")
        ad2_m = build_attmat(adst2_d, f2, "ad2m")

        # W1 @ A (contraction over f1=64): lhsT = W1^T via PE transpose.
        w1t_ps = psum1.tile([f1, in_ch], F32, tag="prep_t")
        nc.tensor.transpose(out=w1t_ps[:], in_=w1_sb[:], identity=ident[:])
        w1t = const.tile([f1, in_ch], F32)
        nc.vector.tensor_copy(out=w1t[:], in_=w1t_ps[:])
        w1as_ps = psum1.tile([in_ch, 4], F32, tag="prep_a")
        w1ad_ps = psum1.tile([in_ch, 4], F32, tag="prep_b")
        nc.tensor.matmul(out=w1as_ps[:], lhsT=w1t[:], rhs=as1_m[0][:], start=True, stop=True)
        nc.tensor.matmul(out=w1ad_ps[:], lhsT=w1t[:], rhs=ad1_m[0][:], start=True, stop=True)

        # w1aug_e (bf16): [W1 | W1@Asrc1] for the per-edge projection
        w1aug_f = const.tile([in_ch, f1 + 4], F32)
        nc.vector.tensor_copy(out=w1aug_f[:, 0:f1], in_=w1_sb[:])
        nc.vector.tensor_copy(out=w1aug_f[:, f1 : f1 + 4], in_=w1as_ps[:])
        w1aug_e = const.tile([in_ch, f1 + 4], BF16)
        nc.vector.tensor_copy(out=w1aug_e[:], in_=w1aug_f[:])
        wad1_bf = const.tile([in_ch, 4], BF16)
        nc.vector.tensor_copy(out=wad1_bf[:], in_=w1ad_ps[:])

        # W2 @ A2 (contraction over f2=256, split into K=128 halves).
        w2as_ps = psum1.tile([f1, 4], F32, tag="prep_a")
        w2ad_ps = psum1.tile([f1, 4], F32, tag="prep_b")
        nkh = f2 // P
        for kh in range(nkh):
            w2t_ps = psum1.tile([P, f1], F32, tag="prep_t")
            nc.tensor.transpose(
                out=w2t_ps[:], in_=w2_sb[:, kh * P : (kh + 1) * P],
                identity=ident[0:f1, 0:f1],
            )
            w2t = pool.tile([P, f1], F32, tag="w2t_sb")
            nc.vector.tensor_copy(out=w2t[:], in_=w2t_ps[:])
            nc.tensor.matmul(
                out=w2as_ps[:], lhsT=w2t[:], rhs=as2_m[kh][:],
                start=(kh == 0), stop=(kh == nkh - 1),
            )
            nc.tensor.matmul(
                out=w2ad_ps[:], lhsT=w2t[:], rhs=ad2_m[kh][:],
                start=(kh == 0), stop=(kh == nkh - 1),
            )
        # waug2p: [I_64 | W2@Asrc2] (rhs for table-2 row building)
        waug2p = const.tile([f1, f1 + 4], F32)
        nc.vector.tensor_copy(out=waug2p[:, 0:f1], in_=ident[0:f1, 0:f1])
        nc.vector.tensor_copy(out=waug2p[:, f1 : f1 + 4], in_=w2as_ps[:])
        wad2_sb = const.tile([f1, 4], F32)
        nc.vector.tensor_copy(out=wad2_sb[:], in_=w2ad_ps[:])
        waug2p_bf = const.tile([f1, f1 + 4], BF16)
        nc.vector.tensor_copy(out=waug2p_bf[:], in_=waug2p[:])
        psum1_cm.__exit__(None, None, None)

        psum = ctx.enter_context(tc.tile_pool(name="psum", bufs=2, space="PSUM"))
        psumo = ctx.enter_context(tc.tile_pool(name="psumo", bufs=1, space="PSUM"))
        psumt = ctx.enter_context(tc.tile_pool(name="psumt", bufs=1, space="PSUM"))

        # persistent per-own-node a_dst values (both layers)
        adst1_own = const.tile([P, ng, 4], F32)
        adst2_own = const.tile([P, ng, 4], F32)

        # ---- P1own: layer-1 a_dst for own nodes ----
        QO = 4
        for g0 in range(0, ng, QO):
            q = min(QO, ng - g0)
            xo = pool.tile([in_ch, QO * P], BF16, tag="p1o_x")
            nc.sync.dma_start(out=xo[:, : q * P], in_=x_ownT_d[:, g0 * P : (g0 + q) * P])
            for k in range(q):
                ps = psum.tile([P, 4], F32, tag="s4")
                nc.tensor.matmul(
                    out=ps[:], lhsT=xo[:, k * P : (k + 1) * P], rhs=wad1_bf[:],
                    start=True, stop=True,
                )
                nc.vector.tensor_copy(out=adst1_own[:, g0 + k, :], in_=ps[:])

        def edge_weights(t_sb, adst_own, g, sg, tag):
            """w = exp(leaky_relu(a_src + a_dst)) per (lane, slot, head);
            den[lane, head] accumulates the softmax denominators."""
            w_sb = pool.tile([P, smax, 4], F32, tag=f"w{tag}")
            den = pool.tile([P, 4], F32, tag=f"den{tag}")
            adst_g = adst_own[:, g, :]
            adst_bc = bass.AP(
                tensor=adst_g.tensor,
                offset=adst_g.offset,
                ap=[adst_g.ap[0], [0, sg], adst_g.ap[1]],
            )
            nc.vector.tensor_tensor(
                out=w_sb[:, :sg, :],
                in0=t_sb[:, :sg, f1 : f1 + 4],
                in1=adst_bc,
                op=mybir.AluOpType.add,
            )
            nc.vector.scalar_tensor_tensor(
                out=w_sb[:, :sg, :],
                in0=w_sb[:, :sg, :],
                scalar=NEG_SLOPE,
                in1=w_sb[:, :sg, :],
                op0=mybir.AluOpType.mult,
                op1=mybir.AluOpType.max,
            )
            for h in range(HEADS):
                nc.scalar.activation(
                    out=w_sb[:, :sg, h],
                    in_=w_sb[:, :sg, h],
                    func=mybir.ActivationFunctionType.Exp,
                    accum_out=den[:, h : h + 1],
                )
            return w_sb, den

        # ---- A1: per-edge projection + layer-1 aggregation -> out1t ----
        with tc.tile_pool(name="gather1", bufs=2) as gpool1:
            for g in range(ng):
                sg = int(s_g[g])
                base = int(s0[g])
                xe = gpool1.tile([in_ch, smax * P], BF16, tag="xe")
                nc.sync.dma_start(
                    out=xe[:, : sg * P],
                    in_=x_edgeT_d[:, base * P : (base + sg) * P],
                )
                t_sb = gpool1.tile([P, smax, f1 + 4], F32, tag="t1")
                for j in range(sg):
                    ps = psum.tile([P, f1 + 4], F32, tag="e1")
                    nc.tensor.matmul(
                        out=ps[:], lhsT=xe[:, j * P : (j + 1) * P], rhs=w1aug_e[:],
                        start=True, stop=True,
                    )
                    nc.vector.tensor_copy(out=t_sb[:, j, :], in_=ps[:])
                msl = maskp_sb[:, base : base + sg]
                mbc = bass.AP(
                    tensor=msl.tensor,
                    offset=msl.offset,
                    ap=[msl.ap[0], msl.ap[1], [0, 4]],
                )
                nc.vector.tensor_tensor(
                    out=t_sb[:, :sg, f1 : f1 + 4],
                    in0=t_sb[:, :sg, f1 : f1 + 4],
                    in1=mbc,
                    op=mybir.AluOpType.add,
                )

                w_sb, den = edge_weights(t_sb, adst1_own, g, sg, "1")
                hv = t_sb[:, :sg, 0:f1].rearrange("p g (h c) -> p g h c", h=HEADS)
                nc.vector.tensor_tensor(
                    out=hv,
                    in0=hv,
                    in1=w_sb[:, :sg, :].to_broadcast([P, sg, HEADS, f1 // HEADS]),
                    op=mybir.AluOpType.mult,
                )
                u = pool.tile([P, f1], F32, tag="u1")
                nc.vector.tensor_reduce(
                    out=u[:],
                    in_=t_sb[:, :sg, 0:f1].rearrange("p g f -> p f g"),
                    axis=mybir.AxisListType.X,
                    op=mybir.AluOpType.add,
                )
                r = pool.tile([P, 4], F32, tag="r1")
                nc.vector.tensor_scalar_add(out=r[:], in0=den[:], scalar1=EPS)
                nc.vector.reciprocal(out=r[:], in_=r[:])

                # finish1: normalize + bias + relu -> out1t; also a_dst2
                o = pool.tile([P, f1], F32, tag="o1")
                nc.vector.tensor_tensor(
                    out=o[:].rearrange("p (h c) -> p h c", h=HEADS),
                    in0=u[:].rearrange("p (h c) -> p h c", h=HEADS),
                    in1=r[:].to_broadcast([P, HEADS, f1 // HEADS]),
                    op=mybir.AluOpType.mult,
                )
                nc.vector.tensor_add(out=o[:], in0=o[:], in1=b1_b[:])
                nc.vector.tensor_scalar_max(out=o[:], in0=o[:], scalar1=0.0)
                tps = psumt.tile([f1, P], F32, tag="tr_ps")
                nc.tensor.transpose(out=tps[:], in_=o[:], identity=ident[:])
                ot = pool.tile([f1, P], F32, tag="o1t")
                nc.vector.tensor_copy(out=ot[:], in_=tps[:])
                nc.sync.dma_start(out=out1t[:, g * P : (g + 1) * P], in_=ot[:])
                psa = psum.tile([P, 4], F32, tag="s4")
                nc.tensor.matmul(
                    out=psa[:], lhsT=ot[:], rhs=wad2_sb[:], start=True, stop=True
                )
                nc.vector.tensor_copy(out=adst2_own[:, g, :], in_=psa[:])

        # ---- AG: exchange layer-1 outputs ----
        nc.gpsimd.collective_compute(
            "AllGather",
            mybir.AluOpType.bypass,
            replica_groups=[list(range(NC))],
            ins=[out1t[:]],
            outs=[ag_out[:]],
        )

        # ---- P2: table-2 rows [h1relu | a_src2] for ALL nodes ----
        QMAX = 4
        for c_i in range(NC):
            for g0 in range(0, ng, QMAX):
                q = min(QMAX, ng - g0)
                lt = pool.tile([f1, QMAX * P], F32, tag="p2_l")
                nc.sync.dma_start(
                    out=lt[:, : q * P],
                    in_=ag_out[c_i * f1 : (c_i + 1) * f1, g0 * P : (g0 + q) * P],
                )
                lt_bf = pool.tile([f1, QMAX * P], BF16, tag="p2_lb")
                nc.scalar.activation(
                    out=lt_bf[:, : q * P], in_=lt[:, : q * P],
                    func=mybir.ActivationFunctionType.Relu,
                )
                hc = pool.tile([P, QMAX, f1 + 4], F32, tag="p2_hc")
                for k in range(q):
                    ps = psum.tile([P, f1 + 4], F32, tag="e1")
                    nc.tensor.matmul(
                        out=ps[:], lhsT=lt_bf[:, k * P : (k + 1) * P], rhs=waug2p_bf[:],
                        start=True, stop=True,
                    )
                    nc.vector.tensor_copy(out=hc[:, k, :], in_=ps[:])
                r0 = (c_i * ng + g0) * P
                nc.sync.dma_start(
                    out=h2cat[r0 : r0 + q * P, :].rearrange("(k p) f -> p k f", k=q),
                    in_=hc[:, :q, :],
                )

        # ---- A2: layer-2 aggregation (gather 68-col rows, 4-head reduce) ----
        with tc.tile_pool(name="gather2", bufs=2) as gpool2:
            for g in range(ng):
                sg = int(s_g[g])
                base = int(s0[g])
                t2 = gpool2.tile([P, smax, f1 + 4], F32, tag="t2")
                for j in range(sg):
                    nc.gpsimd.indirect_dma_start(
                        out=t2[:, j, :],
                        out_offset=None,
                        in_=h2cat[:],
                        in_offset=bass.IndirectOffsetOnAxis(
                            ap=idx_sb[:, base + j : base + j + 1], axis=0
                        ),
                    )
                w_sb, den = edge_weights(t2, adst2_own, g, sg, "2")
                u = pool.tile([P, HEADS, f1], F32, tag="u2")
                for h in range(HEADS):
                    pr = pool.tile([P, smax, f1], F32, tag="pr2")
                    wsl = w_sb[:, :sg, h : h + 1]
                    whb = bass.AP(
                        tensor=wsl.tensor,
                        offset=wsl.offset,
                        ap=[wsl.ap[0], wsl.ap[1], [0, f1]],
                    )
                    nc.vector.tensor_tensor(
                        out=pr[:, :sg, :],
                        in0=t2[:, :sg, 0:f1],
                        in1=whb,
                        op=mybir.AluOpType.mult,
                    )
                    nc.vector.tensor_reduce(
                        out=u[:, h, :],
                        in_=pr[:, :sg, :].rearrange("p g f -> p f g"),
                        axis=mybir.AxisListType.X,
                        op=mybir.AluOpType.add,
                    )
                # finish2: scale by recip(den)/HEADS, apply W2, +b2
                r = pool.tile([P, 4], F32, tag="r2")
                nc.vector.tensor_scalar_add(out=r[:], in0=den[:], scalar1=EPS)
                nc.vector.reciprocal(out=r[:], in_=r[:])
                nc.vector.tensor_scalar_mul(out=r[:], in0=r[:], scalar1=1.0 / HEADS)
                agg = pool.tile([P, HEADS, f1], F32, tag="agg2")
                rb = bass.AP(
                    tensor=r[:].tensor,
                    offset=r[:].offset,
                    ap=[r[:].ap[0], r[:].ap[1], [0, f1]],
                )
                nc.vector.tensor_tensor(
                    out=agg[:], in0=u[:], in1=rb, op=mybir.AluOpType.mult
                )
                aggf = agg[:].rearrange("p h f -> p (h f)")
                ops = psumo.tile([P, out_ch], F32, tag="o2ps")
                for half in range(2):
                    tph = psumt.tile([P, P], F32, tag="tr2")
                    nc.tensor.transpose(
                        out=tph[:],
                        in_=aggf[:, half * P : (half + 1) * P],
                        identity=ident[:],
                    )
                    aggT = pool.tile([P, P], F32, tag=f"aggT{half}")
                    nc.vector.tensor_copy(out=aggT[:], in_=tph[:])
                    nc.tensor.matmul(
                        out=ops[:],
                        lhsT=aggT[:],
                        rhs=(w2r_lo if half == 0 else w2r_hi)[:],
                        start=(half == 0),
                        stop=(half == 1),
                    )
                o2 = pool.tile([P, out_ch], F32, tag="o2")
                nc.vector.tensor_tensor(
                    out=o2[:], in0=ops[:], in1=b2_b[:], op=mybir.AluOpType.add
                )
                nc.sync.dma_start(out=out2_d[g * P : (g + 1) * P, :], in_=o2[:])

    _bass_rust.generate_event_semaphores(nc)
    return nc


def kernel(x, edge_index, W1, att_src1, att_dst1, b1, W2, att_src2, att_dst2, b2, trace=False):
    x = np.asarray(x, dtype=np.float32)
    edge_index = np.asarray(edge_index)
    in_ch = x.shape[1]
    hid = np.asarray(att_src1).shape[1]
    out_ch = np.asarray(att_src2).shape[1]
    f1, f2 = HEADS * hid, HEADS * out_ch

    x_edgeT, maskP, x_ownT, idx_arr, meta = _host_prep(x, edge_index)
    nc = _build_program(meta, in_ch, hid, out_ch)

    common = {
        "w1": np.asarray(W1, dtype=np.float32),
        "asrc1": np.asarray(att_src1, dtype=np.float32).reshape(1, f1),
        "adst1": np.asarray(att_dst1, dtype=np.float32).reshape(1, f1),
        "b1": np.asarray(b1, dtype=np.float32).reshape(1, f1),
        "w2": np.asarray(W2, dtype=np.float32),
        "asrc2": np.asarray(att_src2, dtype=np.float32).reshape(1, f2),
        "adst2": np.asarray(att_dst2, dtype=np.float32).reshape(1, f2),
        "b2": np.asarray(b2, dtype=np.float32).reshape(1, out_ch),
    }
    in_maps = [
        {
            **common,
            "x_edgeT": x_edgeT[c],
            "maskp": maskP[c],
            "x_ownT": x_ownT[c],
            "idx": np.ascontiguousarray(idx_arr[c]),
        }
        for c in range(NC)
    ]
    if trace:
        import axon_prof

        axon_prof.install()
    r = run_bass_kernel_spmd(nc, in_maps, list(range(NC)), trace=trace)

    n, nloc, order = meta["n"], meta["nloc"], meta["order"]
    out = np.zeros((n, out_ch), dtype=np.float32)
    for c in range(NC):
        j = np.arange(nloc)
        rk = j * NC + c
        valid = rk < n
        out[order[rk[valid]]] = r.results[c]["out2"][valid]
    if trace:
        return out, r
    return out
